# revision 11
# baseline (speedup 1.0000x reference)
"""TRN2 Bass kernel for nn_BilinearInteraction — v2 (exact-780 layout).

out[b,k] = sum_{e,f} E[b,i,e] W[k,e,f] E[b,j,f], 780 pairs (i<j), 40 fields.

Layout: slot == pair index k (i-major, j ascending) — no host gather.
W packed by residue r=i%4: r<3 share columns (rows 0/32/64); r=3 gets a
dedicated region at base partition 64, K=64 (rows 64:96 zero, W in 96:128)
since PE base partition must be 0/32/64.

Per core (8 b-tiles of 128):
- PE: per-i PSUM tiles [128,48,32] fp32 (3 banks, bufs=2), matmul pieces
  split at 16-slot (one-bank) marks so each matmul owns its bank.
- Act: one copy per i PSUM fp32 -> SBUF fp16 (the only evictor).
- DVE: m = u16 * e16n (2x); binary tree over f (2x). Everything DVE.
- GpSimd: BANNED (shares SBUF ports with DVE; concurrency collapses both).
- Tree runs per i-aligned segment (~192 slots) into fp16 obt.
"""

import numpy as np

import concourse.bass as bass
import concourse.mybir as mybir
import concourse.tile as tile
from concourse import bacc
from concourse.bass_utils import run_bass_kernel_spmd

NF = 40
E = 32
NPAIR = 780
BATCH = 8192
NCORES = 8
B_CORE = BATCH // NCORES
NBT = B_CORE // 128            # 8 b-tiles per core
GRP = 2                        # b-tiles per instruction group
VCAP = 32                      # psum slots per vp (per btl: 4KB = 2 banks)

# ---- pair/slot tables: slot == k (i-major) ----
IOFF = np.zeros(NF, np.int64)          # first slot of field i
for _i in range(1, NF):
    IOFF[_i] = IOFF[_i - 1] + (NF - _i)
NP_I = [NF - 1 - _i for _i in range(NF)]   # pairs per i

# wp column offsets: residues 0-2 share columns (rows 0/32/64); residue 3
# dedicated region at R3OFF (rows 96:128, read with base partition 64, K=64).
WOFF = np.zeros(NF, np.int64)
_res_tot = [0, 0, 0, 0]
for _i in range(NF - 1):
    WOFF[_i] = _res_tot[_i % 4]
    _res_tot[_i % 4] += NP_I[_i]
R3OFF = max(_res_tot[:3])               # 210
WL = R3OFF + _res_tot[3]                # 390

# virtual pieces: split each i into <=VCAP-slot chunks (psum bank-exact)
VPS = []                                # (i, a, b) local pair ranges
for _i in range(NF - 1):
    _a = 0
    while _a < NP_I[_i]:
        _b = min(_a + 32, NP_I[_i])
        VPS.append((_i, _a, _b))
        _a = _b

# i-aligned tree segments of ~192 slots
SEGS = []                               # (i_first, i_last, slot_lo, slot_hi)
_lo = 0
_i0 = 0
for _i in range(NF - 1):
    hi = int(IOFF[_i]) + NP_I[_i]
    if hi - _lo >= 192 or _i == NF - 2:
        SEGS.append((_i0, _i, _lo, hi))
        _lo = hi
        _i0 = _i + 1
SEG_OF_I = {}
for _s, (_ia, _ib, _l, _h) in enumerate(SEGS):
    for _i in range(_ia, _ib + 1):
        SEG_OF_I[_i] = _s
SEGMAX = max(h - l for (_, _, l, h) in SEGS)


# ---------------- host packing ----------------
def _pack_w(W):
    Wp = np.zeros((128, WL, E), np.float32)
    k = 0
    for i in range(NF - 1):
        r = i % 4
        col = (WOFF[i] if r < 3 else R3OFF + WOFF[i])
        row = 32 * r
        for j in range(i + 1, NF):
            Wp[row:row + 32, col + (j - i - 1), :] = W[k]
            k += 1
    return Wp


def _pack_et(emb):
    et = emb.reshape(NCORES, B_CORE, NF // 4, 4, E).transpose(0, 3, 4, 2, 1)
    return np.ascontiguousarray(et.reshape(NCORES, 128, NF // 4, B_CORE))


# ---------------- bass program ----------------
_CACHED = None


def _build():
    global _CACHED
    if _CACHED is not None:
        return _CACHED

    nc = bacc.Bacc("TRN2", target_bir_lowering=False, debug=False)
    f16 = mybir.dt.float16
    f32 = mybir.dt.float32

    et16_d = nc.dram_tensor("et16", [128, NF // 4, B_CORE], f16,
                            kind="ExternalInput")
    wp_d = nc.dram_tensor("wp", [128, WL, E], f16, kind="ExternalInput")
    e16n_d = nc.dram_tensor("e16n", [NBT, 128, NF, E], f16,
                            kind="ExternalInput")
    o_d = nc.dram_tensor("o", [NBT, 128, NPAIR], f16, kind="ExternalOutput")

    with tile.TileContext(nc) as tc:
        with (
            tc.tile_pool(name="consts", bufs=1) as consts,
            tc.tile_pool(name="en", bufs=2) as en,
            tc.tile_pool(name="u16p", bufs=8) as u16p,
            tc.tile_pool(name="msegp", bufs=2) as msegp,
            tc.tile_pool(name="tree", bufs=1) as tree,
            tc.tile_pool(name="outs", bufs=2) as outs,
            tc.tile_pool(name="upsum", bufs=4, space="PSUM") as upsum,
        ):
            wp_sb = consts.tile([128, WL, E], f16)
            et16_sb = consts.tile([128, NF // 4, B_CORE], f16)
            e16n_g0 = en.tile([128, GRP, NF, E], f16, tag="e16n")

            def _wp_dma(c):       # 53-col chunk c
                s = 53 * c
                e = min(s + 53, WL)
                nc.sync.dma_start(out=wp_sb[:, s:e, :], in_=wp_d[:, s:e, :])

            def _et_dma(m):
                nc.sync.dma_start(out=et16_sb[:, m, :], in_=et16_d[:, m, :])

            # first-need order: i=0..2 use wp c0/et m0; i=3 (r=3) needs
            # cols 210.. = chunks 3-4; then c1 (i=4..12), c5 (r3 i>=7),
            # c2, c6, c7; et16 m_k first used at i=4k.
            # split first-needed loads across DMA queues (one queue is only
            # ~22.5 GB/s; a 320KB single-queue DMA takes ~14us)
            for s in range(0, B_CORE, 256):
                nc.sync.dma_start(out=et16_sb[:, 0, s:s + 256],
                                  in_=et16_d[:, 0, s:s + 256])
            nc.sync.dma_start(out=wp_sb[:, 0:27, :], in_=wp_d[:, 0:27, :])
            nc.sync.dma_start(out=wp_sb[:, 27:53, :], in_=wp_d[:, 27:53, :])
            for btl in range(GRP):
                for jl in (0, 20):
                    nc.sync.dma_start(
                        out=e16n_g0[:, btl, jl:jl + 20, :],
                        in_=e16n_d[btl, :, jl:jl + 20, :])
            for c in (3, 4):
                _wp_dma(c)
            _et_dma(1)
            _wp_dma(1)
            _wp_dma(5)
            _et_dma(2)
            _wp_dma(2)
            _et_dma(3)
            for c in (6, 7):
                _wp_dma(c)
            for m in range(4, NF // 4):
                _et_dma(m)

            for g in range(NBT // GRP):
                bt0 = g * GRP
                if g == 0:
                    e16n = e16n_g0
                else:
                    e16n = en.tile([128, GRP, NF, E], f16, tag="e16n")
                    for btl in range(GRP):
                        nc.sync.dma_start(out=e16n[:, btl, :, :],
                                          in_=e16n_d[bt0 + btl, :, :, :])
                obt = outs.tile([128, GRP, NPAIR], f16, tag="obt")

                mseg = None
                for (i, a, b) in VPS:
                    sidx = SEG_OF_I[i]
                    (ia, ib, slo, shi) = SEGS[sidx]
                    if i == ia and a == 0:
                        mseg = msegp.tile([128, GRP, SEGMAX, E], f16,
                                          tag="mseg")
                    npi = b - a
                    r = i % 4
                    kb = 32 * r if r < 3 else 64
                    ke = kb + (32 if r < 3 else 64)
                    col = (WOFF[i] if r < 3 else R3OFF + WOFF[i]) + a

                    # 16-slot psum tiles (2 banks each, bufs=4): deeper
                    # PE->Act pipeline; two copies fill one 32-slot u16 tile
                    # so the DVE mult count is unchanged.
                    u16 = u16p.tile([128, GRP, VCAP, E], f16, tag="u16")
                    gg = 0
                    while gg < npi:
                        ng = min(16, npi - gg)
                        u_ps = upsum.tile([128, GRP, 16, E], f32, tag="u")
                        for btl in range(GRP):
                            bs = bass.ts(bt0 + btl, 128)
                            nc.tensor.matmul(
                                u_ps[:, btl, :ng, :],
                                et16_sb[kb:ke, i // 4, bs],
                                wp_sb[kb:ke, col + gg:col + gg + ng, :],
                                start=True,
                                stop=True,
                            )
                        nc.scalar.copy(out=u16[:, :, gg:gg + ng, :],
                                       in_=u_ps[:, :, :ng, :])
                        gg += ng
                    mlo = int(IOFF[i]) + a - slo
                    nc.vector.tensor_mul(
                        mseg[:, :, mlo:mlo + npi, :],
                        u16[:, :, :npi, :],
                        e16n[:, :, i + 1 + a:i + 1 + b, :],
                    )

                    if i == ib and b == NP_I[i]:   # segment complete -> tree
                        n = shi - slo
                        s1 = tree.tile([128, GRP, SEGMAX, 16], f16, tag="s1")
                        s2 = tree.tile([128, GRP, SEGMAX, 8], f16, tag="s2")
                        s3 = tree.tile([128, GRP, SEGMAX, 4], f16, tag="s3")
                        s4 = tree.tile([128, GRP, SEGMAX, 2], f16, tag="s4")
                        nc.vector.tensor_add(
                            s1[:, :, :n, :], mseg[:, :, :n, 0:16],
                            mseg[:, :, :n, 16:32])
                        nc.vector.tensor_add(
                            s2[:, :, :n, :], s1[:, :, :n, 0:8],
                            s1[:, :, :n, 8:16])
                        nc.vector.tensor_add(
                            s3[:, :, :n, :], s2[:, :, :n, 0:4],
                            s2[:, :, :n, 4:8])
                        nc.vector.tensor_add(
                            s4[:, :, :n, :], s3[:, :, :n, 0:2],
                            s3[:, :, :n, 2:4])
                        with nc.allow_low_precision(reason="fp16 2-term add"):
                            nc.vector.tensor_add(
                                obt[:, :, slo:shi],
                                s4[:, :, :n, 0], s4[:, :, :n, 1])

                for btl in range(GRP):
                    nc.sync.dma_start(out=o_d[bt0 + btl, :, :],
                                      in_=obt[:, btl, :])

    nc.compile()
    _CACHED = nc
    return nc


# ---------------- public entry ----------------
def _run(embeddings, W, **spmd_kwargs):
    embeddings = np.ascontiguousarray(np.asarray(embeddings, dtype=np.float32))
    W = np.ascontiguousarray(np.asarray(W, dtype=np.float32))

    et16 = _pack_et(embeddings).astype(np.float16)
    e16n = np.ascontiguousarray(
        embeddings.reshape(NCORES, NBT, 128, NF, E).astype(np.float16))
    wp = _pack_w(W).astype(np.float16)

    nc = _build()
    in_maps = [
        {"et16": et16[c], "wp": wp, "e16n": e16n[c]}
        for c in range(NCORES)
    ]
    res = run_bass_kernel_spmd(nc, in_maps, list(range(NCORES)), **spmd_kwargs)

    out = np.empty((BATCH, NPAIR), np.float32)
    for c in range(NCORES):
        o = res.results[c]["o"].reshape(B_CORE, NPAIR)
        out[c * B_CORE:(c + 1) * B_CORE] = o.astype(np.float32)
    return out, res


def kernel(embeddings, W):
    out, _ = _run(embeddings, W)
    return out
